# revision 1
# baseline (speedup 1.0000x reference)
"""MoE (top-2, 8 experts) Trainium2 kernel — expert parallelism across 8 NeuronCores.

Strategy:
  * Host (numpy, fp32, bit-matching the reference's routing): gate logits ->
    softmax -> top-2 -> group token/expert pairs by expert.
  * Core e gets expert e's tokens, transposed to [D, C] (C = padded max token
    count over experts, identical on all cores -> SPMD), plus expert e's
    weights, all split hi/lo into bf16 pairs.
  * Device: two-layer FFN, each matmul computed as 3 bf16 passes
    (hi*hi + hi*lo + lo*hi) accumulating into fp32 PSUM — ~1e-5 relative
    accuracy at 3x the bf16 matmul rate (4x cheaper than native fp32 mode
    would allow... native fp32 is 4 cycles/row, this is 3).
    L1: hT[H, C] = relu(W1^T xT + b1); split hi/lo on device.
    L2: yT[D, C] = W2^T hT (bias b2 and gate probs applied on host).
  * Host: out[tok] += (yT[:, :c_e]^T + b2[e]) * p_e.

Self-contained: hardcodes shapes from the problem spec (B=4, S=1024, D=1024,
H=2048, E=8, top-2), computes routing capacity from the actual inputs.
"""

import contextlib
import ctypes
import sys
import types

import numpy as np
import ml_dtypes

B, S, D, H, E, TOPK = 4, 1024, 1024, 2048, 8, 2
N_CORES = 8
P = 128
KO1 = D // P   # 8  K-tiles in layer 1
M1 = H // P    # 16 M-tiles in layer 1
KO2 = H // P   # 16 K-tiles in layer 2
M2 = D // P    # 8  M-tiles in layer 2
NT = 512       # moving-dim (token) tile width; PSUM fp32 bank limit

BF16 = ml_dtypes.bfloat16


def _install_axon_ntff_hook():
    """This image's antenv lacks axon_hooks; inject the ctypes NTFF profiling
    hook so run_bass_kernel_spmd(trace=True) works instead of crashing."""
    try:
        import antenv.axon_hooks  # noqa: F401
        return
    except ImportError:
        pass
    try:
        import antenv
    except ImportError:
        return

    so_path = "/opt/axon/libaxon_pjrt.so"
    try:
        lib = ctypes.CDLL(so_path)
    except OSError:
        lib = None
    hook = None
    if lib is not None and hasattr(lib, "axon_start_nrt_profile"):
        lib.axon_start_nrt_profile.argtypes = [
            ctypes.POINTER(ctypes.c_int64),
            ctypes.c_size_t,
        ]
        lib.axon_start_nrt_profile.restype = ctypes.c_int64
        lib.axon_stop_nrt_profile.argtypes = [ctypes.c_char_p]
        lib.axon_stop_nrt_profile.restype = ctypes.c_int64

        @contextlib.contextmanager
        def hook(output_dir, device_ids):
            import jax

            jax.devices()
            if device_ids:
                ids = (ctypes.c_int64 * len(device_ids))(*device_ids)
                rc = lib.axon_start_nrt_profile(ids, len(device_ids))
            else:
                rc = lib.axon_start_nrt_profile(None, 0)
            if rc != 0:
                raise RuntimeError(f"axon_start_nrt_profile rc={rc}")
            try:
                yield
            finally:
                n = lib.axon_stop_nrt_profile(str(output_dir).encode())
                print(f"profile: {n} file(s) -> {output_dir}", file=sys.stderr)

    mod = types.ModuleType("antenv.axon_hooks")
    state = {"hook": hook}
    mod.set_axon_ntff_profile_hook = lambda h: state.__setitem__("hook", h)
    mod.get_axon_ntff_profile_hook = lambda: state["hook"]
    sys.modules["antenv.axon_hooks"] = mod
    antenv.axon_hooks = mod


def _patch_upload_artifacts():
    """Trace post-processing uploads artifacts to S3; make failures non-fatal."""
    from concourse import bass_utils

    orig = bass_utils.upload_artifacts
    if getattr(orig, "_moe_safe", False):
        return

    def safe_upload(tmpdir):
        try:
            return orig(tmpdir)
        except Exception:
            return f"file://{tmpdir}"

    safe_upload._moe_safe = True
    bass_utils.upload_artifacts = safe_upload


def _chunks(C):
    out = []
    c0 = 0
    while c0 < C:
        w = min(NT, C - c0)
        out.append((c0, w))
        c0 += w
    return out


_PROGRAM_CACHE = {}


def _build_program(C):
    """Per-core bass program: 2-layer FFN on [D, C] tokens, 3-pass bf16 hi/lo."""
    import concourse.tile as tile
    from concourse import bacc, mybir

    nc = bacc.Bacc(None, debug=False)
    bf = mybir.dt.bfloat16
    f32 = mybir.dt.float32

    xhi_d = nc.dram_tensor("xhi", [D, C], bf, kind="ExternalInput")
    xlo_d = nc.dram_tensor("xlo", [D, C], bf, kind="ExternalInput")
    w1hi_d = nc.dram_tensor("w1hi", [D, H], bf, kind="ExternalInput")
    w1lo_d = nc.dram_tensor("w1lo", [D, H], bf, kind="ExternalInput")
    w2hi_d = nc.dram_tensor("w2hi", [H, D], bf, kind="ExternalInput")
    w2lo_d = nc.dram_tensor("w2lo", [H, D], bf, kind="ExternalInput")
    b1_d = nc.dram_tensor("b1r", [P, M1], f32, kind="ExternalInput")
    yT_d = nc.dram_tensor("yT", [D, C], f32, kind="ExternalOutput")

    chunks = _chunks(C)

    with tile.TileContext(nc) as tc:
        with (
            tc.tile_pool(name="wpool", bufs=2) as wpool,
            tc.tile_pool(name="xpool", bufs=1) as xpool,
            tc.tile_pool(name="hpool", bufs=1) as hpool,
            tc.tile_pool(name="hfpool", bufs=4) as hfpool,
            tc.tile_pool(name="ypool", bufs=4) as ypool,
            tc.tile_pool(name="bpool", bufs=1) as bpool,
            tc.tile_pool(name="pspool", bufs=4, space="PSUM") as pspool,
        ):
            b1_sb = bpool.tile([P, M1], f32, tag="b1")
            nc.sync.dma_start(out=b1_sb, in_=b1_d[:, :])

            # Weights layer 1 (the wpool slots are recycled for w2 afterwards).
            w1hi = wpool.tile([P, KO1, H], bf, tag="w")
            w1lo = wpool.tile([P, KO1, H], bf, tag="w")
            nc.sync.dma_start(out=w1hi, in_=w1hi_d.rearrange("(ko p) h -> p ko h", p=P))
            nc.sync.dma_start(out=w1lo, in_=w1lo_d.rearrange("(ko p) h -> p ko h", p=P))

            xhi = xpool.tile([P, KO1, C], bf, tag="xhi")
            xlo = xpool.tile([P, KO1, C], bf, tag="xlo")
            nc.sync.dma_start(out=xhi, in_=xhi_d.rearrange("(ko p) c -> p ko c", p=P))
            nc.sync.dma_start(out=xlo, in_=xlo_d.rearrange("(ko p) c -> p ko c", p=P))

            hhi = hpool.tile([P, KO2, C], bf, tag="hhi")
            hlo = hpool.tile([P, KO2, C], bf, tag="hlo")

            # ---- Layer 1: hT = relu(W1^T @ xT + b1), split hi/lo ----
            for c0, w in chunks:
                csl = slice(c0, c0 + w)
                for m in range(M1):
                    msl = slice(m * P, (m + 1) * P)
                    ps = pspool.tile([P, NT], f32, tag="ps")
                    for ko in range(KO1):
                        nc.tensor.matmul(
                            ps[:, :w], w1hi[:, ko, msl], xhi[:, ko, csl],
                            start=(ko == 0), stop=False,
                        )
                        nc.tensor.matmul(
                            ps[:, :w], w1hi[:, ko, msl], xlo[:, ko, csl],
                            start=False, stop=False,
                        )
                        nc.tensor.matmul(
                            ps[:, :w], w1lo[:, ko, msl], xhi[:, ko, csl],
                            start=False, stop=(ko == KO1 - 1),
                        )
                    hf = hfpool.tile([P, NT], f32, tag="hf")
                    nc.scalar.activation(
                        out=hf[:, :w], in_=ps[:, :w],
                        func=mybir.ActivationFunctionType.Relu,
                        bias=b1_sb[:, m : m + 1], scale=1.0,
                    )
                    nc.vector.tensor_copy(out=hhi[:, m, csl], in_=hf[:, :w])
                    nc.vector.tensor_sub(hlo[:, m, csl], hf[:, :w], hhi[:, m, csl])

            # ---- Layer 2: yT = W2^T @ hT ----
            w2hi = wpool.tile([P, KO2, D], bf, tag="w")
            w2lo = wpool.tile([P, KO2, D], bf, tag="w")
            nc.sync.dma_start(out=w2hi, in_=w2hi_d.rearrange("(ko p) d -> p ko d", p=P))
            nc.sync.dma_start(out=w2lo, in_=w2lo_d.rearrange("(ko p) d -> p ko d", p=P))

            for c0, w in chunks:
                csl = slice(c0, c0 + w)
                for m in range(M2):
                    msl = slice(m * P, (m + 1) * P)
                    ps = pspool.tile([P, NT], f32, tag="ps")
                    for ko in range(KO2):
                        nc.tensor.matmul(
                            ps[:, :w], w2hi[:, ko, msl], hhi[:, ko, csl],
                            start=(ko == 0), stop=False,
                        )
                        nc.tensor.matmul(
                            ps[:, :w], w2hi[:, ko, msl], hlo[:, ko, csl],
                            start=False, stop=False,
                        )
                        nc.tensor.matmul(
                            ps[:, :w], w2lo[:, ko, msl], hhi[:, ko, csl],
                            start=False, stop=(ko == KO2 - 1),
                        )
                    yt = ypool.tile([P, NT], f32, tag="y")
                    nc.vector.tensor_copy(out=yt[:, :w], in_=ps[:, :w])
                    nc.sync.dma_start(out=yT_d[msl, csl], in_=yt[:, :w])

    nc.finalize()
    return nc


def _split_hilo(a):
    hi = np.asarray(a, np.float32).astype(BF16)
    lo = (a - hi.astype(np.float32)).astype(BF16)
    return hi, lo


LAST_EXEC_NS = None
LAST_TRACE = None


def kernel(x, Wg, W1, b1, W2, b2):
    import os

    global LAST_EXEC_NS, LAST_TRACE

    _install_axon_ntff_hook()
    _patch_upload_artifacts()
    from concourse.bass_utils import run_bass_kernel_spmd

    x = np.asarray(x, np.float32)
    Wg = np.asarray(Wg, np.float32)
    W1 = np.asarray(W1, np.float32)
    b1 = np.asarray(b1, np.float32)
    W2 = np.asarray(W2, np.float32)
    b2 = np.asarray(b2, np.float32)

    N = B * S
    xm = np.ascontiguousarray(x.reshape(N, D))

    # --- host routing: identical math to the reference (fp32) ---
    logits = xm @ Wg
    mx = logits.max(-1, keepdims=True)
    ex = np.exp(logits - mx)
    probs = ex / ex.sum(-1, keepdims=True)
    idx = np.argsort(-probs, axis=-1, kind="stable")[:, :TOPK]  # top-2, desc
    p2 = np.take_along_axis(probs, idx, axis=-1)

    toks_per_e = []
    probs_per_e = []
    for e in range(E):
        toks, slots = np.where(idx == e)
        toks_per_e.append(toks)
        probs_per_e.append(p2[toks, slots])
    counts = np.array([len(t) for t in toks_per_e])
    C = max(P, int(-(-counts.max() // P) * P))  # round up to 128

    # --- per-core inputs ---
    xmT = np.ascontiguousarray(xm.T)  # [D, N]
    in_maps = []
    for e in range(E):
        toks = toks_per_e[e]
        xsT = np.zeros((D, C), np.float32)
        xsT[:, : len(toks)] = xmT[:, toks]
        xhi, xlo = _split_hilo(xsT)
        w1hi, w1lo = _split_hilo(W1[e])
        w2hi, w2lo = _split_hilo(W2[e])
        b1r = np.ascontiguousarray(b1[e].reshape(M1, P).T)  # [128, 16]
        in_maps.append({
            "xhi": xhi, "xlo": xlo,
            "w1hi": w1hi, "w1lo": w1lo,
            "w2hi": w2hi, "w2lo": w2lo,
            "b1r": b1r,
        })

    if C not in _PROGRAM_CACHE:
        _PROGRAM_CACHE[C] = _build_program(C)
    nc = _PROGRAM_CACHE[C]

    trace = bool(int(os.environ.get("BASS_MOE_TRACE", "0")))
    kw = {}
    if trace:
        kw["trace"] = True
        tdir = os.environ.get("BASS_MOE_TRACE_DIR")
        if tdir:
            kw["tmpdir"] = tdir
    res = run_bass_kernel_spmd(nc, in_maps, core_ids=list(range(N_CORES)), **kw)
    LAST_EXEC_NS = res.exec_time_ns
    LAST_TRACE = res.instructions_and_trace[1] if res.instructions_and_trace else None

    # --- host combine: bias2 + gates + scatter-add ---
    out = np.zeros((N, D), np.float32)
    for e in range(E):
        toks = toks_per_e[e]
        if len(toks) == 0:
            continue
        y = res.results[e]["yT"][:, : len(toks)].T  # [c_e, D]
        out[toks] += (y + b2[e]) * probs_per_e[e][:, None]
    return out.reshape(B, S, D)
